# revision 14
# baseline (speedup 1.0000x reference)
"""RoPE + ALiBi single-head attention (B=8, T=2048, H=256) on 8 Trainium2
cores, batch-parallel (one batch element per core).

v2: bf16 matmul operands (enables fast-weight-load; halves DMA/DVE/SBUF
traffic), PE warm-up matmuls so the HAM clock gate is at 2.4 GHz before the
real GEMMs start, GEMM2/denominator matmuls interleaved into GEMM1's
ACT-paced slots so the PE never stalls on the exp stream, and the
denominator ones-matmuls halved via DVE pair-sums of adjacent at tiles.

Per-core algorithm (all compute on device):
  qeT/keT = RoPE(qT/kT)                     [DVE, bf16, pipelined with the
                                             input DMA in 512-col chunks]
  scoresT[s,t] = sum_d keT[d,s]*qeT[d,t]    [PE bf16, 2 k-tiles, fp32 PSUM]
  at[s,t] = exp(scoresT*scale + slope*s)    [ACT, PSUM->SBUF bf16]
     (the -slope*t alibi term is constant per softmax column and cancels)
  den[t] = sum_s at[s,t]                    [DVE pair-adds + 8 accumulating
                                             ones-matmuls into [1,512] PSUM]
  outT[h,t] = (sum_s v[s,h]*at[s,t]) / den  [PE bf16; reciprocal via magic
                                             bit-trick + 2 Newton steps,
                                             broadcast on GpSimd, DVE mul]
Host only reshapes/transposes/casts and precomputes rope/alibi tables.
"""
import math
from contextlib import ExitStack

import numpy as np
from ml_dtypes import bfloat16

import concourse.bacc as bacc
import concourse.tile as tile
from concourse import mybir
from concourse.bass_utils import run_bass_kernel_spmd

B, T, H = 8, 2048, 256
HALF = H // 2          # 128 (rope half, also partition dim)
NCHUNK = 4
CHUNK = T // NCHUNK    # 512 query columns per chunk
NS = T // 128          # 16 key tiles
ROPE_BASE = 10000.0
SLOPE = 2.0 ** (-8.0)
SCALE = 1.0 / math.sqrt(H)
RECIP_MAGIC = 0x7EF127EA  # fast fp32 reciprocal seed: magic - bits(x)
NWARM = 22             # junk matmuls to lift the PE HAM gate to 2.4 GHz

F32 = mybir.dt.float32
BF16 = mybir.dt.bfloat16
I32 = mybir.dt.int32
EXP = mybir.ActivationFunctionType.Exp
MULT = mybir.AluOpType.mult
ADD = mybir.AluOpType.add

TRACE = False           # test harness sets True for NTFF profiling
LAST_RESULTS = None     # BassKernelResults of the last run (for profiling)

_NC_CACHE = {}


def _build_nc():
    nc = bacc.Bacc("TRN2", target_bir_lowering=False, debug=False)
    qt_d = nc.dram_tensor("qt", [H, T], BF16, kind="ExternalInput").ap()
    kt_d = nc.dram_tensor("kt", [H, T], BF16, kind="ExternalInput").ap()
    vt_d = nc.dram_tensor("vt", [128, NS * H], BF16, kind="ExternalInput").ap()
    cos_d = nc.dram_tensor("costab", [HALF, T], BF16, kind="ExternalInput").ap()
    sin_d = nc.dram_tensor("sintab", [HALF, T], BF16, kind="ExternalInput").ap()
    bias_d = nc.dram_tensor("alibi", [128, NS], F32, kind="ExternalInput").ap()
    ot_d = nc.dram_tensor("ot", [H, T], F32, kind="ExternalOutput").ap()

    with tile.TileContext(nc) as tc, ExitStack() as ctx:
        const = ctx.enter_context(tc.tile_pool(name="const", bufs=1))
        rpool = ctx.enter_context(tc.tile_pool(name="ropeout", bufs=1))
        vpool = ctx.enter_context(tc.tile_pool(name="vpool", bufs=1))
        stage = ctx.enter_context(tc.tile_pool(name="stage", bufs=1))
        atp = ctx.enter_context(tc.tile_pool(name="atp", bufs=36))
        smp = ctx.enter_context(tc.tile_pool(name="smp", bufs=10))
        dn = ctx.enter_context(tc.tile_pool(name="dn", bufs=2))
        onp = ctx.enter_context(tc.tile_pool(name="onp", bufs=4))
        ps1p = ctx.enter_context(tc.tile_pool(name="ps1", bufs=3, space="PSUM"))
        ps2p = ctx.enter_context(tc.tile_pool(name="ps2", bufs=4, space="PSUM"))
        pdnp = ctx.enter_context(tc.tile_pool(name="pdn", bufs=1, space="PSUM"))

        # small constants: alibi bias (gpsimd queue), ones column for the
        # denominator partition-reduce matmuls, reciprocal magic row, junk
        # operand for the PE warm-up matmuls
        junkw = const.tile([128, CHUNK], BF16)
        nc.vector.memset(junkw[:], 0.0)
        biasb = const.tile([128, NS], F32)
        nc.gpsimd.dma_start(biasb[:], bias_d[:])
        ones_b = const.tile([128, 1], BF16)
        nc.vector.memset(ones_b[:], 1.0)
        magicb = const.tile([1, CHUNK], I32)
        nc.vector.memset(magicb[:], RECIP_MAGIC)
        # preload the exp activation table while the PE is still warming up
        tpre = dn.tile([1, 8], F32, tag="tpre")
        nc.scalar.activation(tpre[:], biasb[0:1, 0:8], EXP)

        # PE warm-up: one long accumulation group of junk matmuls (no
        # per-MM semaphores) runs while the DMAs and the first rope chunks
        # are in flight, flipping the HAM clock gate to 8/8 before the
        # first real GEMM issues
        junk_ps = ps1p.tile([128, CHUNK], F32, tag="p1", name="junk_ps")
        for i in range(NWARM):
            nc.tensor.matmul(junk_ps[:], junkw[:, 0:128], junkw[:],
                             start=(i == 0), stop=(i == NWARM - 1))

        # persistent bf16 operands for the two GEMMs
        qe = [rpool.tile([128, T], BF16, name=f"qe{i}", tag=f"qe{i}")
              for i in range(2)]
        ke = [rpool.tile([128, T], BF16, name=f"ke{i}", tag=f"ke{i}")
              for i in range(2)]
        vr = vpool.tile([128, NS * H], BF16)

        # full-width staging tiles, filled by per-chunk DMAs (subtile deps
        # let rope/GEMM1 start as soon as their columns land)
        cosb = stage.tile([128, T], BF16, tag="cosb")
        sinb = stage.tile([128, T], BF16, tag="sinb")
        ks0 = stage.tile([128, T], BF16, tag="ks0")
        ks1 = stage.tile([128, T], BF16, tag="ks1")
        qs0 = stage.tile([128, T], BF16, tag="qs0")
        qs1 = stage.tile([128, T], BF16, tag="qs1")

        # spread input DMA descriptor issue across four engine queues so
        # the chunk-0 transfers all start as early as possible
        def load_cols(cc):
            col = slice(cc * CHUNK, (cc + 1) * CHUNK)
            nc.sync.dma_start(ks0[:, col], kt_d[0:128, col])
            nc.sync.dma_start(ks1[:, col], kt_d[128:256, col])
            if cc == 0:
                nc.scalar.dma_start(cosb[:, col], cos_d[:, col])
                nc.sync.dma_start(sinb[:, col], sin_d[:, col])
            else:
                nc.gpsimd.dma_start(cosb[:, col], cos_d[:, col])
                nc.gpsimd.dma_start(sinb[:, col], sin_d[:, col])

        def load_q_cols(cc, eng):
            col = slice(cc * CHUNK, (cc + 1) * CHUNK)
            eng.dma_start(qs0[:, col], qt_d[0:128, col])
            eng.dma_start(qs1[:, col], qt_d[128:256, col])

        def rope(src0, src1, dst, col, tmptag):
            """dst0[:,col] = s0*cos - s1*sin ; dst1[:,col] = s1*cos + s0*sin"""
            n = col.stop - col.start
            nc.vector.tensor_mul(dst[0][:, col], src0[:, col], cosb[:, col])
            tmp = stage.tile([128, n], BF16, tag="rtmp", bufs=3,
                             name=f"tmp{tmptag}{col.start}")
            nc.vector.tensor_mul(tmp[:], src1[:, col], sinb[:, col])
            nc.vector.tensor_sub(dst[0][:, col], dst[0][:, col], tmp[:])
            nc.vector.tensor_mul(dst[1][:, col], src1[:, col], cosb[:, col])
            tmp2 = stage.tile([128, n], BF16, tag="rtmp", bufs=3,
                              name=f"tmp2{tmptag}{col.start}")
            nc.vector.tensor_mul(tmp2[:], src0[:, col], sinb[:, col])
            nc.vector.tensor_add(dst[1][:, col], dst[1][:, col], tmp2[:])

        # chunk-0 inputs first, then k/q rope pipelined with remaining DMAs
        load_cols(0)
        load_q_cols(0, nc.scalar)
        # v arrives pre-tiled [128, s*256+h] bf16 from the host (gpsimd
        # queue, needed once GEMM2 starts ~15us in)
        for s in range(0, NS, 8):
            nc.gpsimd.dma_start(vr[:, s * H:(s + 8) * H],
                                vt_d[:, s * H:(s + 8) * H])
        for cc in range(1, NCHUNK):
            load_cols(cc)
            load_q_cols(cc, nc.gpsimd)
        rope(ks0, ks1, ke, slice(0, CHUNK), "k0")
        rope(qs0, qs1, qe, slice(0, CHUNK), "q0")
        for cc in range(1, NCHUNK):
            rope(ks0, ks1, ke, slice(cc * CHUNK, (cc + 1) * CHUNK), f"k{cc}")

        mm = nc.tensor.matmul

        def g2_slot(at_tiles, p2, s):
            for h in range(2):
                mm(p2[h][:], vr[:, s * H + h * 128: s * H + (h + 1) * 128],
                   at_tiles[s][:], start=(s == 0), stop=(s == NS - 1))

        def normalize(c, p2, recipb):
            tcol = slice(c * CHUNK, (c + 1) * CHUNK)
            for h in range(2):
                on = onp.tile([128, CHUNK], F32, tag="on", name=f"on{c}_{h}")
                nc.vector.tensor_mul(on[:], p2[h][:], recipb[:])
                nc.sync.dma_start(ot_d[h * 128:(h + 1) * 128, tcol], on[:])

        prev = None  # (p2, recipb) of the previous chunk, normalized inside
                     # the next chunk's slot stream (keeps the in-order PE
                     # and DVE queues from stalling on cross-engine waits)
        for c in range(NCHUNK):
            tcol = slice(c * CHUNK, (c + 1) * CHUNK)
            if c + 1 < NCHUNK:
                # rope next chunk's q columns ahead of its GEMM1
                rope(qs0, qs1, qe, slice((c + 1) * CHUNK, (c + 2) * CHUNK),
                     f"q{c + 1}")
            at_tiles = []
            pairs = []
            pden = pdnp.tile([1, CHUNK], F32)
            p2 = [ps2p.tile([128, CHUNK], F32, tag="p2", name=f"p2_{c}_{h}")
                  for h in range(2)]
            for s in range(NS):
                p1 = ps1p.tile([128, CHUNK], F32, tag="p1", name=f"p1_{c}_{s}")
                mm(p1[:], ke[0][:, s * 128:(s + 1) * 128], qe[0][:, tcol],
                   start=True, stop=False)
                mm(p1[:], ke[1][:, s * 128:(s + 1) * 128], qe[1][:, tcol],
                   start=False, stop=True)
                # fill GEMM1's exp-paced slots with this chunk's GEMM2 (one
                # s-tile behind the activation stream) and the accumulating
                # denominator matmuls
                if s >= 1:
                    g2_slot(at_tiles, p2, s - 1)
                if c == 0:
                    # chunk 0: DVE is busy roping k, so the denominator sums
                    # at tiles directly (no pair-add dependency)
                    if s >= 2:
                        mm(pden[:], ones_b[:], at_tiles[s - 2][:],
                           start=(s == 2), stop=False)
                elif s >= 4 and s % 2 == 0:
                    j = s // 2 - 2
                    mm(pden[:], ones_b[:], pairs[j][:],
                       start=(j == 0), stop=False)
                if c == 0 and s in (5, 9):
                    # keep the PE/HAM warm while GEMM1 is rope-paced
                    for _ in range(2):
                        mm(junk_ps[:], junkw[:, 0:128], junkw[:],
                           start=True, stop=True)
                if s == 4 and prev is not None:
                    # broadcast prev chunk's 1/den across partitions with a
                    # rank-1 matmul (emitted here so the in-order PE queue
                    # has its dependency long satisfied)
                    p2p, r_prev, recp_prev = prev
                    mm(recp_prev[:], ones_rr[:], r_prev[:],
                       start=True, stop=True)
                if s == 6 and prev is not None:
                    normalize(c - 1, prev[0], prev[2])
                at = atp.tile([128, CHUNK], BF16, tag="at")
                nc.scalar.activation(at[:], p1[:], EXP,
                                     bias=biasb[:, s:s + 1], scale=SCALE)
                at_tiles.append(at)
                if c > 0 and s % 2 == 1 and s < NS - 1:
                    pr = smp.tile([128, CHUNK], BF16, tag="pair",
                                  name=f"pair{c}_{s // 2}")
                    nc.vector.tensor_add(pr[:], at_tiles[s - 1][:],
                                         at_tiles[s][:])
                    pairs.append(pr)
            g2_slot(at_tiles, p2, NS - 1)
            # last two at tiles go into the denominator directly (no DVE
            # pair-add on the critical tail)
            if c > 0:
                mm(pden[:], ones_b[:], pairs[6][:], start=False, stop=False)
            mm(pden[:], ones_b[:], at_tiles[NS - 2][:], start=False, stop=False)
            mm(pden[:], ones_b[:], at_tiles[NS - 1][:], start=False, stop=True)

            # reciprocal of the [1, CHUNK] denominator row:
            # seed r = bits(magic - bits(d)), then one Newton step
            den_sb = dn.tile([1, CHUNK], F32, tag="den_sb")
            nc.vector.tensor_copy(den_sb[:], pden[0:1, :])
            r = dn.tile([1, CHUNK], F32, tag="rA", name=f"rA{c}")
            nc.vector.tensor_sub(r[:].bitcast(I32), magicb[:],
                                 den_sb[:].bitcast(I32))
            t2 = dn.tile([1, CHUNK], F32, tag="nt", bufs=2, name=f"nt{c}")
            nc.vector.scalar_tensor_tensor(t2[:], den_sb[:], -1.0, r[:],
                                           MULT, MULT)
            r_new = dn.tile([1, CHUNK], mybir.dt.float32r, tag="r0", bufs=2,
                            name=f"r{c}")
            nc.vector.scalar_tensor_tensor(r_new[:], t2[:], 2.0, r[:],
                                           ADD, MULT)
            recp = ps1p.tile([128, CHUNK], F32, tag="p1", name=f"recp{c}")
            prev = (p2, r_new, recp)

        p2p, r_prev, recp_prev = prev
        mm(recp_prev[:], ones_rr[:], r_prev[:], start=True, stop=True)
        normalize(NCHUNK - 1, p2p, recp_prev)

    nc.compile()
    return nc


def _get_nc():
    if "nc" not in _NC_CACHE:
        _NC_CACHE["nc"] = _build_nc()
    return _NC_CACHE["nc"]


def _tables():
    j = np.arange(HALF, dtype=np.float64)
    inv = ROPE_BASE ** (-2.0 * j / H)
    t = np.arange(T, dtype=np.float64)
    fr = np.outer(inv, t)                       # [128, T]
    cos = np.cos(fr).astype(bfloat16)
    sin = np.sin(fr).astype(bfloat16)
    p = np.arange(128, dtype=np.float64)[:, None]
    sidx = p + 128.0 * np.arange(NS, dtype=np.float64)[None, :]
    bias = (SLOPE * sidx).astype(np.float32)    # [128, NS]
    return cos, sin, bias


def kernel(q, k, v):
    global LAST_RESULTS
    q = np.asarray(q, dtype=np.float32)
    k = np.asarray(k, dtype=np.float32)
    v = np.asarray(v, dtype=np.float32)
    assert q.shape == (B, T, H), q.shape

    nc = _get_nc()
    cos, sin, bias = _tables()
    in_maps = []
    for b in range(B):
        # vt[p, s*256+h] = v[s*128+p, h]
        vt = np.ascontiguousarray(
            v[b].reshape(NS, 128, H).transpose(1, 0, 2).reshape(128, NS * H)
        ).astype(bfloat16)
        in_maps.append({
            "qt": np.ascontiguousarray(q[b].T).astype(bfloat16),
            "kt": np.ascontiguousarray(k[b].T).astype(bfloat16),
            "vt": vt,
            "costab": cos,
            "sintab": sin,
            "alibi": bias,
        })
    kw = {}
    if TRACE:
        kw = dict(trace=True)
    res = run_bass_kernel_spmd(nc, in_maps, list(range(B)), **kw)
    LAST_RESULTS = res
    out = np.stack(
        [np.ascontiguousarray(res.results[b]["ot"]).T for b in range(B)], axis=0
    )
    return out[None].astype(np.float32)


# revision 15
# speedup vs baseline: 1.0117x; 1.0117x over previous
"""RoPE + ALiBi single-head attention (B=8, T=2048, H=256) on 8 Trainium2
cores, batch-parallel (one batch element per core).

v2: bf16 matmul operands (enables fast-weight-load; halves DMA/DVE/SBUF
traffic), PE warm-up matmuls so the HAM clock gate is at 2.4 GHz before the
real GEMMs start, GEMM2/denominator matmuls interleaved into GEMM1's
ACT-paced slots so the PE never stalls on the exp stream, and the
denominator ones-matmuls halved via DVE pair-sums of adjacent at tiles.

Per-core algorithm (all compute on device):
  qeT/keT = RoPE(qT/kT)                     [DVE, bf16, pipelined with the
                                             input DMA in 512-col chunks]
  scoresT[s,t] = sum_d keT[d,s]*qeT[d,t]    [PE bf16, 2 k-tiles, fp32 PSUM]
  at[s,t] = exp(scoresT*scale + slope*s)    [ACT, PSUM->SBUF bf16]
     (the -slope*t alibi term is constant per softmax column and cancels)
  den[t] = sum_s at[s,t]                    [DVE pair-adds + 8 accumulating
                                             ones-matmuls into [1,512] PSUM]
  outT[h,t] = (sum_s v[s,h]*at[s,t]) / den  [PE bf16; reciprocal via magic
                                             bit-trick + 2 Newton steps,
                                             broadcast on GpSimd, DVE mul]
Host only reshapes/transposes/casts and precomputes rope/alibi tables.
"""
import math
from contextlib import ExitStack

import numpy as np
from ml_dtypes import bfloat16

import concourse.bacc as bacc
import concourse.tile as tile
from concourse import mybir
from concourse.bass_utils import run_bass_kernel_spmd

B, T, H = 8, 2048, 256
HALF = H // 2          # 128 (rope half, also partition dim)
NCHUNK = 4
CHUNK = T // NCHUNK    # 512 query columns per chunk
NS = T // 128          # 16 key tiles
ROPE_BASE = 10000.0
SLOPE = 2.0 ** (-8.0)
SCALE = 1.0 / math.sqrt(H)
RECIP_MAGIC = 0x7EF127EA  # fast fp32 reciprocal seed: magic - bits(x)
NWARM = 30             # junk matmuls to lift the PE HAM gate to 2.4 GHz

F32 = mybir.dt.float32
BF16 = mybir.dt.bfloat16
I32 = mybir.dt.int32
EXP = mybir.ActivationFunctionType.Exp
MULT = mybir.AluOpType.mult
ADD = mybir.AluOpType.add

TRACE = False           # test harness sets True for NTFF profiling
LAST_RESULTS = None     # BassKernelResults of the last run (for profiling)

_NC_CACHE = {}


def _build_nc():
    nc = bacc.Bacc("TRN2", target_bir_lowering=False, debug=False)
    qt_d = nc.dram_tensor("qt", [H, T], BF16, kind="ExternalInput").ap()
    kt_d = nc.dram_tensor("kt", [H, T], BF16, kind="ExternalInput").ap()
    vt_d = nc.dram_tensor("vt", [128, NS * H], BF16, kind="ExternalInput").ap()
    cos_d = nc.dram_tensor("costab", [HALF, T], BF16, kind="ExternalInput").ap()
    sin_d = nc.dram_tensor("sintab", [HALF, T], BF16, kind="ExternalInput").ap()
    bias_d = nc.dram_tensor("alibi", [128, NS], F32, kind="ExternalInput").ap()
    ot_d = nc.dram_tensor("ot", [H, T], F32, kind="ExternalOutput").ap()

    with tile.TileContext(nc) as tc, ExitStack() as ctx:
        const = ctx.enter_context(tc.tile_pool(name="const", bufs=1))
        rpool = ctx.enter_context(tc.tile_pool(name="ropeout", bufs=1))
        vpool = ctx.enter_context(tc.tile_pool(name="vpool", bufs=1))
        stage = ctx.enter_context(tc.tile_pool(name="stage", bufs=1))
        atp = ctx.enter_context(tc.tile_pool(name="atp", bufs=36))
        smp = ctx.enter_context(tc.tile_pool(name="smp", bufs=10))
        dn = ctx.enter_context(tc.tile_pool(name="dn", bufs=2))
        onp = ctx.enter_context(tc.tile_pool(name="onp", bufs=4))
        ps1p = ctx.enter_context(tc.tile_pool(name="ps1", bufs=3, space="PSUM"))
        ps2p = ctx.enter_context(tc.tile_pool(name="ps2", bufs=4, space="PSUM"))
        pdnp = ctx.enter_context(tc.tile_pool(name="pdn", bufs=1, space="PSUM"))

        # small constants: alibi bias (gpsimd queue), ones column for the
        # denominator partition-reduce matmuls, reciprocal magic row, junk
        # operand for the PE warm-up matmuls
        junkw = const.tile([128, CHUNK], BF16)
        nc.vector.memset(junkw[:], 0.0)
        biasb = const.tile([128, NS], F32)
        nc.gpsimd.dma_start(biasb[:], bias_d[:])
        ones_b = const.tile([128, 1], BF16)
        nc.vector.memset(ones_b[:], 1.0)
        magicb = const.tile([1, CHUNK], I32)
        nc.vector.memset(magicb[:], RECIP_MAGIC)
        # preload the exp activation table while the PE is still warming up
        tpre = dn.tile([1, 8], F32, tag="tpre")
        nc.scalar.activation(tpre[:], biasb[0:1, 0:8], EXP)

        # PE warm-up: one long accumulation group of junk matmuls (no
        # per-MM semaphores) runs while the DMAs and the first rope chunks
        # are in flight, flipping the HAM clock gate to 8/8 before the
        # first real GEMM issues
        junk_ps = ps1p.tile([128, CHUNK], F32, tag="p1", name="junk_ps")
        for i in range(NWARM):
            nc.tensor.matmul(junk_ps[:], junkw[:, 0:128], junkw[:],
                             start=(i == 0), stop=(i == NWARM - 1))

        # persistent bf16 operands for the two GEMMs
        qe = [rpool.tile([128, T], BF16, name=f"qe{i}", tag=f"qe{i}")
              for i in range(2)]
        ke = [rpool.tile([128, T], BF16, name=f"ke{i}", tag=f"ke{i}")
              for i in range(2)]
        vr = vpool.tile([128, NS * H], BF16)

        # full-width staging tiles, filled by per-chunk DMAs (subtile deps
        # let rope/GEMM1 start as soon as their columns land)
        cosb = stage.tile([128, T], BF16, tag="cosb")
        sinb = stage.tile([128, T], BF16, tag="sinb")
        ks0 = stage.tile([128, T], BF16, tag="ks0")
        ks1 = stage.tile([128, T], BF16, tag="ks1")
        qs0 = stage.tile([128, T], BF16, tag="qs0")
        qs1 = stage.tile([128, T], BF16, tag="qs1")

        # spread input DMA descriptor issue across four engine queues so
        # the chunk-0 transfers all start as early as possible
        def load_cols(cc):
            col = slice(cc * CHUNK, (cc + 1) * CHUNK)
            nc.sync.dma_start(ks0[:, col], kt_d[0:128, col])
            nc.sync.dma_start(ks1[:, col], kt_d[128:256, col])
            if cc == 0:
                nc.scalar.dma_start(cosb[:, col], cos_d[:, col])
                nc.sync.dma_start(sinb[:, col], sin_d[:, col])
            else:
                nc.gpsimd.dma_start(cosb[:, col], cos_d[:, col])
                nc.gpsimd.dma_start(sinb[:, col], sin_d[:, col])

        def load_q_cols(cc, eng):
            col = slice(cc * CHUNK, (cc + 1) * CHUNK)
            eng.dma_start(qs0[:, col], qt_d[0:128, col])
            eng.dma_start(qs1[:, col], qt_d[128:256, col])

        def rope(src0, src1, dst, col, tmptag):
            """dst0[:,col] = s0*cos - s1*sin ; dst1[:,col] = s1*cos + s0*sin"""
            n = col.stop - col.start
            nc.vector.tensor_mul(dst[0][:, col], src0[:, col], cosb[:, col])
            tmp = stage.tile([128, n], BF16, tag="rtmp", bufs=3,
                             name=f"tmp{tmptag}{col.start}")
            nc.vector.tensor_mul(tmp[:], src1[:, col], sinb[:, col])
            nc.vector.tensor_sub(dst[0][:, col], dst[0][:, col], tmp[:])
            nc.vector.tensor_mul(dst[1][:, col], src1[:, col], cosb[:, col])
            tmp2 = stage.tile([128, n], BF16, tag="rtmp", bufs=3,
                              name=f"tmp2{tmptag}{col.start}")
            nc.vector.tensor_mul(tmp2[:], src0[:, col], sinb[:, col])
            nc.vector.tensor_add(dst[1][:, col], dst[1][:, col], tmp2[:])

        # chunk-0 inputs first, then k/q rope pipelined with remaining DMAs
        load_cols(0)
        load_q_cols(0, nc.scalar)
        # v arrives pre-tiled [128, s*256+h] bf16 from the host (gpsimd
        # queue, needed once GEMM2 starts ~15us in)
        for s in range(0, NS, 8):
            nc.gpsimd.dma_start(vr[:, s * H:(s + 8) * H],
                                vt_d[:, s * H:(s + 8) * H])
        for cc in range(1, NCHUNK):
            load_cols(cc)
            load_q_cols(cc, nc.gpsimd)
        rope(qs0, qs1, qe, slice(0, CHUNK), "q0")
        rope(ks0, ks1, ke, slice(0, CHUNK), "k0")
        for cc in range(1, NCHUNK):
            rope(ks0, ks1, ke, slice(cc * CHUNK, (cc + 1) * CHUNK), f"k{cc}")

        mm = nc.tensor.matmul

        def g2_slot(at_tiles, p2, s):
            for h in range(2):
                mm(p2[h][:], vr[:, s * H + h * 128: s * H + (h + 1) * 128],
                   at_tiles[s][:], start=(s == 0), stop=(s == NS - 1))

        def normalize(c, p2, recipb):
            tcol = slice(c * CHUNK, (c + 1) * CHUNK)
            for h in range(2):
                on = onp.tile([128, CHUNK], F32, tag="on", name=f"on{c}_{h}")
                nc.vector.tensor_mul(on[:], p2[h][:], recipb[:])
                nc.sync.dma_start(ot_d[h * 128:(h + 1) * 128, tcol], on[:])

        prev = None  # (p2, recipb) of the previous chunk, normalized inside
                     # the next chunk's slot stream (keeps the in-order PE
                     # and DVE queues from stalling on cross-engine waits)
        for c in range(NCHUNK):
            tcol = slice(c * CHUNK, (c + 1) * CHUNK)
            if c + 1 < NCHUNK:
                # rope next chunk's q columns ahead of its GEMM1
                rope(qs0, qs1, qe, slice((c + 1) * CHUNK, (c + 2) * CHUNK),
                     f"q{c + 1}")
            at_tiles = []
            pairs = []
            pden = pdnp.tile([1, CHUNK], F32)
            p2 = [ps2p.tile([128, CHUNK], F32, tag="p2", name=f"p2_{c}_{h}")
                  for h in range(2)]
            for s in range(NS):
                p1 = ps1p.tile([128, CHUNK], F32, tag="p1", name=f"p1_{c}_{s}")
                mm(p1[:], ke[0][:, s * 128:(s + 1) * 128], qe[0][:, tcol],
                   start=True, stop=False)
                mm(p1[:], ke[1][:, s * 128:(s + 1) * 128], qe[1][:, tcol],
                   start=False, stop=True)
                # fill GEMM1's exp-paced slots with this chunk's GEMM2 (one
                # s-tile behind the activation stream) and the accumulating
                # denominator matmuls
                if s >= 1:
                    g2_slot(at_tiles, p2, s - 1)
                if c == 0:
                    # chunk 0: DVE is busy roping k, so the denominator sums
                    # at tiles directly (no pair-add dependency)
                    if s >= 2:
                        mm(pden[:], ones_b[:], at_tiles[s - 2][:],
                           start=(s == 2), stop=False)
                elif s >= 4 and s % 2 == 0:
                    j = s // 2 - 2
                    mm(pden[:], ones_b[:], pairs[j][:],
                       start=(j == 0), stop=False)
                if c == 0 and s in (5, 9):
                    # keep the PE/HAM warm while GEMM1 is rope-paced
                    for _ in range(2):
                        mm(junk_ps[:], junkw[:, 0:128], junkw[:],
                           start=True, stop=True)
                if s == 4 and prev is not None:
                    # broadcast prev chunk's 1/den across partitions with a
                    # rank-1 matmul (emitted here so the in-order PE queue
                    # has its dependency long satisfied)
                    p2p, r_prev, recp_prev = prev
                    mm(recp_prev[:], ones_rr[:], r_prev[:],
                       start=True, stop=True)
                if s == 6 and prev is not None:
                    normalize(c - 1, prev[0], prev[2])
                at = atp.tile([128, CHUNK], BF16, tag="at")
                nc.scalar.activation(at[:], p1[:], EXP,
                                     bias=biasb[:, s:s + 1], scale=SCALE)
                at_tiles.append(at)
                if c > 0 and s % 2 == 1 and s < NS - 1:
                    pr = smp.tile([128, CHUNK], BF16, tag="pair",
                                  name=f"pair{c}_{s // 2}")
                    nc.vector.tensor_add(pr[:], at_tiles[s - 1][:],
                                         at_tiles[s][:])
                    pairs.append(pr)
            g2_slot(at_tiles, p2, NS - 1)
            # last two at tiles go into the denominator directly (no DVE
            # pair-add on the critical tail)
            if c > 0:
                mm(pden[:], ones_b[:], pairs[6][:], start=False, stop=False)
            mm(pden[:], ones_b[:], at_tiles[NS - 2][:], start=False, stop=False)
            mm(pden[:], ones_b[:], at_tiles[NS - 1][:], start=False, stop=True)

            # reciprocal of the [1, CHUNK] denominator row:
            # seed r = bits(magic - bits(d)), then one Newton step
            den_sb = dn.tile([1, CHUNK], F32, tag="den_sb")
            nc.vector.tensor_copy(den_sb[:], pden[0:1, :])
            r = dn.tile([1, CHUNK], F32, tag="rA", name=f"rA{c}")
            nc.vector.tensor_sub(r[:].bitcast(I32), magicb[:],
                                 den_sb[:].bitcast(I32))
            t2 = dn.tile([1, CHUNK], F32, tag="nt", bufs=2, name=f"nt{c}")
            nc.vector.scalar_tensor_tensor(t2[:], den_sb[:], -1.0, r[:],
                                           MULT, MULT)
            r_new = dn.tile([1, CHUNK], mybir.dt.float32r, tag="r0", bufs=2,
                            name=f"r{c}")
            nc.vector.scalar_tensor_tensor(r_new[:], t2[:], 2.0, r[:],
                                           ADD, MULT)
            recp = ps1p.tile([128, CHUNK], F32, tag="p1", name=f"recp{c}")
            prev = (p2, r_new, recp)

        p2p, r_prev, recp_prev = prev
        mm(recp_prev[:], ones_rr[:], r_prev[:], start=True, stop=True)
        normalize(NCHUNK - 1, p2p, recp_prev)

    nc.compile()
    return nc


def _get_nc():
    if "nc" not in _NC_CACHE:
        _NC_CACHE["nc"] = _build_nc()
    return _NC_CACHE["nc"]


def _tables():
    j = np.arange(HALF, dtype=np.float64)
    inv = ROPE_BASE ** (-2.0 * j / H)
    t = np.arange(T, dtype=np.float64)
    fr = np.outer(inv, t)                       # [128, T]
    cos = np.cos(fr).astype(bfloat16)
    sin = np.sin(fr).astype(bfloat16)
    p = np.arange(128, dtype=np.float64)[:, None]
    sidx = p + 128.0 * np.arange(NS, dtype=np.float64)[None, :]
    bias = (SLOPE * sidx).astype(np.float32)    # [128, NS]
    return cos, sin, bias


def kernel(q, k, v):
    global LAST_RESULTS
    q = np.asarray(q, dtype=np.float32)
    k = np.asarray(k, dtype=np.float32)
    v = np.asarray(v, dtype=np.float32)
    assert q.shape == (B, T, H), q.shape

    nc = _get_nc()
    cos, sin, bias = _tables()
    in_maps = []
    for b in range(B):
        # vt[p, s*256+h] = v[s*128+p, h]
        vt = np.ascontiguousarray(
            v[b].reshape(NS, 128, H).transpose(1, 0, 2).reshape(128, NS * H)
        ).astype(bfloat16)
        in_maps.append({
            "qt": np.ascontiguousarray(q[b].T).astype(bfloat16),
            "kt": np.ascontiguousarray(k[b].T).astype(bfloat16),
            "vt": vt,
            "costab": cos,
            "sintab": sin,
            "alibi": bias,
        })
    kw = {}
    if TRACE:
        kw = dict(trace=True)
    res = run_bass_kernel_spmd(nc, in_maps, list(range(B)), **kw)
    LAST_RESULTS = res
    out = np.stack(
        [np.ascontiguousarray(res.results[b]["ot"]).T for b in range(B)], axis=0
    )
    return out[None].astype(np.float32)


# revision 16
# speedup vs baseline: 1.0662x; 1.0539x over previous
"""RoPE + ALiBi single-head attention (B=8, T=2048, H=256) on 8 Trainium2
cores, batch-parallel (one batch element per core).

v2: bf16 matmul operands (enables fast-weight-load; halves DMA/DVE/SBUF
traffic), PE warm-up matmuls so the HAM clock gate is at 2.4 GHz before the
real GEMMs start, GEMM2/denominator matmuls interleaved into GEMM1's
ACT-paced slots so the PE never stalls on the exp stream, and the
denominator ones-matmuls halved via DVE pair-sums of adjacent at tiles.

Per-core algorithm (all compute on device):
  qeT/keT = RoPE(qT/kT)                     [DVE, bf16, pipelined with the
                                             input DMA in 512-col chunks]
  scoresT[s,t] = sum_d keT[d,s]*qeT[d,t]    [PE bf16, 2 k-tiles, fp32 PSUM]
  at[s,t] = exp(scoresT*scale + slope*s)    [ACT, PSUM->SBUF bf16]
     (the -slope*t alibi term is constant per softmax column and cancels)
  den[t] = sum_s at[s,t]                    [DVE pair-adds + 8 accumulating
                                             ones-matmuls into [1,512] PSUM]
  outT[h,t] = (sum_s v[s,h]*at[s,t]) / den  [PE bf16; reciprocal via magic
                                             bit-trick + 2 Newton steps,
                                             broadcast on GpSimd, DVE mul]
Host only reshapes/transposes/casts and precomputes rope/alibi tables.
"""
import math
from contextlib import ExitStack

import numpy as np
from ml_dtypes import bfloat16

import concourse.bacc as bacc
import concourse.tile as tile
from concourse import mybir
from concourse.bass_utils import run_bass_kernel_spmd

B, T, H = 8, 2048, 256
HALF = H // 2          # 128 (rope half, also partition dim)
NCHUNK = 4
CHUNK = T // NCHUNK    # 512 query columns per chunk
NS = T // 128          # 16 key tiles
ROPE_BASE = 10000.0
SLOPE = 2.0 ** (-8.0)
SCALE = 1.0 / math.sqrt(H)
RECIP_MAGIC = 0x7EF127EA  # fast fp32 reciprocal seed: magic - bits(x)
NWARM = 36             # junk matmuls to lift the PE HAM gate to 2.4 GHz

F32 = mybir.dt.float32
BF16 = mybir.dt.bfloat16
I32 = mybir.dt.int32
EXP = mybir.ActivationFunctionType.Exp
MULT = mybir.AluOpType.mult
ADD = mybir.AluOpType.add

TRACE = False           # test harness sets True for NTFF profiling
LAST_RESULTS = None     # BassKernelResults of the last run (for profiling)

_NC_CACHE = {}


def _build_nc():
    nc = bacc.Bacc("TRN2", target_bir_lowering=False, debug=False)
    qt_d = nc.dram_tensor("qt", [H, T], BF16, kind="ExternalInput").ap()
    kt_d = nc.dram_tensor("kt", [H, T], BF16, kind="ExternalInput").ap()
    vt_d = nc.dram_tensor("vt", [128, NS * H], BF16, kind="ExternalInput").ap()
    cos_d = nc.dram_tensor("costab", [HALF, T], BF16, kind="ExternalInput").ap()
    sin_d = nc.dram_tensor("sintab", [HALF, T], BF16, kind="ExternalInput").ap()
    bias_d = nc.dram_tensor("alibi", [128, NS], F32, kind="ExternalInput").ap()
    ot_d = nc.dram_tensor("ot", [H, T], F32, kind="ExternalOutput").ap()

    with tile.TileContext(nc) as tc, ExitStack() as ctx:
        const = ctx.enter_context(tc.tile_pool(name="const", bufs=1))
        rpool = ctx.enter_context(tc.tile_pool(name="ropeout", bufs=1))
        vpool = ctx.enter_context(tc.tile_pool(name="vpool", bufs=1))
        stage = ctx.enter_context(tc.tile_pool(name="stage", bufs=1))
        atp = ctx.enter_context(tc.tile_pool(name="atp", bufs=36))
        smp = ctx.enter_context(tc.tile_pool(name="smp", bufs=10))
        dn = ctx.enter_context(tc.tile_pool(name="dn", bufs=2))
        onp = ctx.enter_context(tc.tile_pool(name="onp", bufs=4))
        ps1p = ctx.enter_context(tc.tile_pool(name="ps1", bufs=3, space="PSUM"))
        ps2p = ctx.enter_context(tc.tile_pool(name="ps2", bufs=4, space="PSUM"))
        pdnp = ctx.enter_context(tc.tile_pool(name="pdn", bufs=1, space="PSUM"))

        # small constants: alibi bias (gpsimd queue), ones column for the
        # denominator partition-reduce matmuls, reciprocal magic row, junk
        # operand for the PE warm-up matmuls
        junkw = const.tile([128, CHUNK], BF16)
        nc.vector.memset(junkw[:], 0.0)
        biasb = const.tile([128, NS], F32)
        nc.gpsimd.dma_start(biasb[:], bias_d[:])
        ones_b = const.tile([128, 1], BF16)
        nc.vector.memset(ones_b[:], 1.0)
        magicb = const.tile([1, CHUNK], I32)
        nc.vector.memset(magicb[:], RECIP_MAGIC)
        # preload the exp activation table while the PE is still warming up
        tpre = dn.tile([1, 8], F32, tag="tpre")
        nc.scalar.activation(tpre[:], biasb[0:1, 0:8], EXP)

        # PE warm-up: one long accumulation group of junk matmuls (no
        # per-MM semaphores) runs while the DMAs and the first rope chunks
        # are in flight, flipping the HAM clock gate to 8/8 before the
        # first real GEMM issues
        junk_ps = ps1p.tile([128, CHUNK], F32, tag="p1", name="junk_ps")
        for i in range(NWARM):
            nc.tensor.matmul(junk_ps[:], junkw[:, 0:128], junkw[:],
                             start=(i == 0), stop=(i == NWARM - 1))

        # persistent bf16 operands for the two GEMMs
        qe = [rpool.tile([128, T], BF16, name=f"qe{i}", tag=f"qe{i}")
              for i in range(2)]
        ke = [rpool.tile([128, T], BF16, name=f"ke{i}", tag=f"ke{i}")
              for i in range(2)]
        vr = vpool.tile([128, NS * H], BF16)

        # full-width staging tiles, filled by per-chunk DMAs (subtile deps
        # let rope/GEMM1 start as soon as their columns land)
        cosb = stage.tile([128, T], BF16, tag="cosb")
        sinb = stage.tile([128, T], BF16, tag="sinb")
        ks0 = stage.tile([128, T], BF16, tag="ks0")
        ks1 = stage.tile([128, T], BF16, tag="ks1")
        qs0 = stage.tile([128, T], BF16, tag="qs0")
        qs1 = stage.tile([128, T], BF16, tag="qs1")

        # spread input DMA descriptor issue across four engine queues so
        # the chunk-0 transfers all start as early as possible
        def load_cols(cc):
            col = slice(cc * CHUNK, (cc + 1) * CHUNK)
            nc.sync.dma_start(ks0[:, col], kt_d[0:128, col])
            nc.sync.dma_start(ks1[:, col], kt_d[128:256, col])
            if cc == 0:
                nc.scalar.dma_start(cosb[:, col], cos_d[:, col])
                nc.sync.dma_start(sinb[:, col], sin_d[:, col])
            else:
                nc.gpsimd.dma_start(cosb[:, col], cos_d[:, col])
                nc.gpsimd.dma_start(sinb[:, col], sin_d[:, col])

        def load_q_cols(cc, eng):
            col = slice(cc * CHUNK, (cc + 1) * CHUNK)
            eng.dma_start(qs0[:, col], qt_d[0:128, col])
            eng.dma_start(qs1[:, col], qt_d[128:256, col])

        def rope(src0, src1, dst, col, tmptag):
            """dst0[:,col] = s0*cos - s1*sin ; dst1[:,col] = s1*cos + s0*sin"""
            n = col.stop - col.start
            nc.vector.tensor_mul(dst[0][:, col], src0[:, col], cosb[:, col])
            tmp = stage.tile([128, n], BF16, tag="rtmp", bufs=3,
                             name=f"tmp{tmptag}{col.start}")
            nc.vector.tensor_mul(tmp[:], src1[:, col], sinb[:, col])
            nc.vector.tensor_sub(dst[0][:, col], dst[0][:, col], tmp[:])
            nc.vector.tensor_mul(dst[1][:, col], src1[:, col], cosb[:, col])
            tmp2 = stage.tile([128, n], BF16, tag="rtmp", bufs=3,
                              name=f"tmp2{tmptag}{col.start}")
            nc.vector.tensor_mul(tmp2[:], src0[:, col], sinb[:, col])
            nc.vector.tensor_add(dst[1][:, col], dst[1][:, col], tmp2[:])

        # chunk-0 inputs first, then k/q rope pipelined with remaining DMAs
        load_cols(0)
        load_q_cols(0, nc.scalar)
        # v arrives pre-tiled [128, s*256+h] bf16 from the host (gpsimd
        # queue, needed once GEMM2 starts ~15us in)
        for s in range(0, NS, 8):
            nc.gpsimd.dma_start(vr[:, s * H:(s + 8) * H],
                                vt_d[:, s * H:(s + 8) * H])
        for cc in range(1, NCHUNK):
            load_cols(cc)
            load_q_cols(cc, nc.gpsimd)
        rope(qs0, qs1, qe, slice(0, CHUNK), "q0")
        rope(ks0, ks1, ke, slice(0, CHUNK), "k0")
        for cc in range(1, NCHUNK):
            rope(ks0, ks1, ke, slice(cc * CHUNK, (cc + 1) * CHUNK), f"k{cc}")

        mm = nc.tensor.matmul

        def g2_slot(at_tiles, p2, s):
            for h in range(2):
                mm(p2[h][:], vr[:, s * H + h * 128: s * H + (h + 1) * 128],
                   at_tiles[s][:], start=(s == 0), stop=(s == NS - 1))

        def normalize(c, p2, recipb):
            tcol = slice(c * CHUNK, (c + 1) * CHUNK)
            for h in range(2):
                on = onp.tile([128, CHUNK], F32, tag="on", name=f"on{c}_{h}")
                nc.vector.tensor_mul(on[:], p2[h][:], recipb[:])
                nc.sync.dma_start(ot_d[h * 128:(h + 1) * 128, tcol], on[:])

        prev = None  # (p2, recipb) of the previous chunk, normalized inside
                     # the next chunk's slot stream (keeps the in-order PE
                     # and DVE queues from stalling on cross-engine waits)
        for c in range(NCHUNK):
            tcol = slice(c * CHUNK, (c + 1) * CHUNK)
            if c + 1 < NCHUNK:
                # rope next chunk's q columns ahead of its GEMM1
                rope(qs0, qs1, qe, slice((c + 1) * CHUNK, (c + 2) * CHUNK),
                     f"q{c + 1}")
            at_tiles = []
            pairs = []
            pden = pdnp.tile([1, CHUNK], F32)
            p2 = [ps2p.tile([128, CHUNK], F32, tag="p2", name=f"p2_{c}_{h}")
                  for h in range(2)]
            for s in range(NS):
                p1 = ps1p.tile([128, CHUNK], F32, tag="p1", name=f"p1_{c}_{s}")
                mm(p1[:], ke[0][:, s * 128:(s + 1) * 128], qe[0][:, tcol],
                   start=True, stop=False)
                mm(p1[:], ke[1][:, s * 128:(s + 1) * 128], qe[1][:, tcol],
                   start=False, stop=True)
                # fill GEMM1's exp-paced slots with this chunk's GEMM2 (one
                # s-tile behind the activation stream) and the accumulating
                # denominator matmuls
                if s >= 1:
                    g2_slot(at_tiles, p2, s - 1)
                if c == NCHUNK - 1 and s >= 4 and s % 2 == 0:
                    # last chunk: spread den matmuls so the softmax
                    # denominator closes right behind the exp stream
                    j = s // 2 - 2
                    mm(pden[:], ones_b[:], pairs[j][:],
                       start=(j == 0), stop=False)
                if c == 0 and s in (5, 9):
                    # keep the PE/HAM warm while GEMM1 is rope-paced
                    for _ in range(2):
                        mm(junk_ps[:], junkw[:, 0:128], junkw[:],
                           start=True, stop=True)
                if s == 4 and prev is not None:
                    # broadcast prev chunk's 1/den across partitions with a
                    # rank-1 matmul (emitted here so the in-order PE queue
                    # has its dependency long satisfied)
                    p2p, r_prev, recp_prev = prev
                    mm(recp_prev[:], ones_rr[:], r_prev[:],
                       start=True, stop=True)
                if s == 6 and prev is not None:
                    normalize(c - 1, prev[0], prev[2])
                at = atp.tile([128, CHUNK], BF16, tag="at")
                nc.scalar.activation(at[:], p1[:], EXP,
                                     bias=biasb[:, s:s + 1], scale=SCALE)
                at_tiles.append(at)
                if c > 0 and s % 2 == 1 and s < NS - 1:
                    pr = smp.tile([128, CHUNK], BF16, tag="pair",
                                  name=f"pair{c}_{s // 2}")
                    nc.vector.tensor_add(pr[:], at_tiles[s - 1][:],
                                         at_tiles[s][:])
                    pairs.append(pr)
            g2_slot(at_tiles, p2, NS - 1)
            # denominator burst: chunks 0-2 sum here (uniform GEMM slots,
            # no mid-stream transitions); the last two at tiles always go
            # in directly (no DVE pair-add on the critical tail)
            if c == 0:
                for j in range(NS - 2):
                    mm(pden[:], ones_b[:], at_tiles[j][:],
                       start=(j == 0), stop=False)
            elif c < NCHUNK - 1:
                for j in range(7):
                    mm(pden[:], ones_b[:], pairs[j][:],
                       start=(j == 0), stop=False)
            else:
                mm(pden[:], ones_b[:], pairs[6][:], start=False, stop=False)
            mm(pden[:], ones_b[:], at_tiles[NS - 2][:], start=False, stop=False)
            mm(pden[:], ones_b[:], at_tiles[NS - 1][:], start=False, stop=True)

            # reciprocal of the [1, CHUNK] denominator row:
            # seed r = bits(magic - bits(d)), then one Newton step
            den_sb = dn.tile([1, CHUNK], F32, tag="den_sb")
            nc.vector.tensor_copy(den_sb[:], pden[0:1, :])
            r = dn.tile([1, CHUNK], F32, tag="rA", name=f"rA{c}")
            nc.vector.tensor_sub(r[:].bitcast(I32), magicb[:],
                                 den_sb[:].bitcast(I32))
            t2 = dn.tile([1, CHUNK], F32, tag="nt", bufs=2, name=f"nt{c}")
            nc.vector.scalar_tensor_tensor(t2[:], den_sb[:], -1.0, r[:],
                                           MULT, MULT)
            r_new = dn.tile([1, CHUNK], mybir.dt.float32r, tag="r0", bufs=2,
                            name=f"r{c}")
            nc.vector.scalar_tensor_tensor(r_new[:], t2[:], 2.0, r[:],
                                           ADD, MULT)
            recp = ps1p.tile([128, CHUNK], F32, tag="p1", name=f"recp{c}")
            prev = (p2, r_new, recp)

        p2p, r_prev, recp_prev = prev
        mm(recp_prev[:], ones_rr[:], r_prev[:], start=True, stop=True)
        normalize(NCHUNK - 1, p2p, recp_prev)

    nc.compile()
    return nc


def _get_nc():
    if "nc" not in _NC_CACHE:
        _NC_CACHE["nc"] = _build_nc()
    return _NC_CACHE["nc"]


def _tables():
    j = np.arange(HALF, dtype=np.float64)
    inv = ROPE_BASE ** (-2.0 * j / H)
    t = np.arange(T, dtype=np.float64)
    fr = np.outer(inv, t)                       # [128, T]
    cos = np.cos(fr).astype(bfloat16)
    sin = np.sin(fr).astype(bfloat16)
    p = np.arange(128, dtype=np.float64)[:, None]
    sidx = p + 128.0 * np.arange(NS, dtype=np.float64)[None, :]
    bias = (SLOPE * sidx).astype(np.float32)    # [128, NS]
    return cos, sin, bias


def kernel(q, k, v):
    global LAST_RESULTS
    q = np.asarray(q, dtype=np.float32)
    k = np.asarray(k, dtype=np.float32)
    v = np.asarray(v, dtype=np.float32)
    assert q.shape == (B, T, H), q.shape

    nc = _get_nc()
    cos, sin, bias = _tables()
    in_maps = []
    for b in range(B):
        # vt[p, s*256+h] = v[s*128+p, h]
        vt = np.ascontiguousarray(
            v[b].reshape(NS, 128, H).transpose(1, 0, 2).reshape(128, NS * H)
        ).astype(bfloat16)
        in_maps.append({
            "qt": np.ascontiguousarray(q[b].T).astype(bfloat16),
            "kt": np.ascontiguousarray(k[b].T).astype(bfloat16),
            "vt": vt,
            "costab": cos,
            "sintab": sin,
            "alibi": bias,
        })
    kw = {}
    if TRACE:
        kw = dict(trace=True)
    res = run_bass_kernel_spmd(nc, in_maps, list(range(B)), **kw)
    LAST_RESULTS = res
    out = np.stack(
        [np.ascontiguousarray(res.results[b]["ot"]).T for b in range(B)], axis=0
    )
    return out[None].astype(np.float32)
